# revision 33
# baseline (speedup 1.0000x reference)
"""Trainium2 Bass kernel for the CoAttnLayer problem.

Computes, per example b (B=16, all dims 1024):
    P   = E @ W^T                      (proj)
    S   = P @ Src^T                    (sim, (e, l))
    edit_w  = softmax_l(S + srcmask)   -> edit_ctx = edit_w @ Src
    src_w   = softmax_e(S + editmask)  -> src_ctx  = src_w^T @ E

Sharding: data-parallel over batch, 2 examples per core on 8 cores;
W^T replicated. All heavy matmuls run in float32r (fp32 operands
truncated to ~fp22 by the PE, full fp32 PSUM accumulation).

Key layout trick: the matrix chains are arranged so every softmax
reduction happens along the free dimension, and the exponentiated
weights are transposed on the PE (128x128 transpose-mode matmuls) into
the layout the following context matmul needs as its stationary operand.

Masking (mask is additive -1e9, exactly like the reference's -inf up to
fp32 underflow of the resulting exp):
  - src mask (over l) for the edit path: the row-max is taken unmasked
    (softmax is shift-invariant so this only rescales exp/Z together;
    the extra scale cancels), masked columns are excluded from the
    context matmul by zeroing the corresponding Src rows on the host,
    and Z is computed with a tiny PE matmul against a 0/1 validity
    column.
  - edit mask (over e) for the src path: added exactly via a rank-1
    (K=1) accumulating matmul appended to the S^T chain, so exp()
    yields exact zeros for masked rows and Z comes from the activation
    engine's accumulate output.
"""

import os
import sys

for _p in ("/opt/trn_rl_repo",):
    if os.path.isdir(_p) and _p not in sys.path:
        sys.path.insert(0, _p)

import numpy as np

import concourse.bass as bass
import concourse.tile as tile
from concourse import mybir
from concourse.bass import ts
from concourse.masks import make_identity
from concourse.bass_utils import run_bass_kernel_spmd

B = 16
L = 1024          # LE = LS = DE = DS = 1024
NCORES = 8
PER_CORE = B // NCORES
P = 128           # partitions
KB = L // P       # 8 k-tiles
N2 = 2            # two 512-wide n-tiles
F32 = mybir.dt.float32
F32R = mybir.dt.float32r
AX = mybir.AxisListType.X
EXP = mybir.ActivationFunctionType.Exp
NEG = -1.0e9


def build_nc(n_examples=PER_CORE, fence=True):
    nc = bass.Bass("TRN2", target_bir_lowering=False)

    et_d = nc.dram_tensor("et", (n_examples, L, L), F32, kind="ExternalInput")     # E^T  (d, e)
    srct_d = nc.dram_tensor("srct", (n_examples, L, L), F32, kind="ExternalInput")  # Src^T (s, l)
    srcz_d = nc.dram_tensor("srcz", (n_examples, L, L), F32, kind="ExternalInput")  # Src, masked rows zeroed (l, s)
    en_d = nc.dram_tensor("en", (n_examples, L, L), F32, kind="ExternalInput")      # E natural (e, d)
    wt_d = nc.dram_tensor("wt", (L, L), F32, kind="ExternalInput")                  # W^T (d, s)
    emcol_d = nc.dram_tensor("emcol", (n_examples, P, KB), F32, kind="ExternalInput")  # additive edit mask [p, eb]
    s01r_d = nc.dram_tensor("s01r", (n_examples, L), F32, kind="ExternalInput")     # src validity 0/1 row
    oe_d = nc.dram_tensor("oe", (n_examples, L, L), F32, kind="ExternalOutput")     # edit_ctx
    os_d = nc.dram_tensor("osr", (n_examples, L, L), F32, kind="ExternalOutput")    # src_ctx

    def load_split(sb_tile, dram, eng=None):
        # one dma_start per k-slice so consumers start on the first slice
        # instead of the whole 4MB tensor; eng picks the HWDGE queue
        e = eng if eng is not None else nc.sync
        for k in range(KB):
            e.dma_start(
                out=sb_tile[:, k, :],
                in_=dram[k * P : (k + 1) * P, :].bitcast(F32R),
            )

    def load_one(sb_tile, dram, eng=None):
        # whole tensor in one DMA dispatch (queue-sequencer time is the
        # scarce resource, not DMA bandwidth)
        e = eng if eng is not None else nc.sync
        e.dma_start(
            out=sb_tile,
            in_=dram.rearrange("(k p) l -> p k l", p=P).bitcast(F32R),
        )

    with tile.TileContext(nc) as tc:
        with (
            tc.tile_pool(name="persist", bufs=1) as persist,
            tc.tile_pool(name="big", bufs=5) as big,
            tc.tile_pool(name="blk", bufs=4) as blk,
            tc.tile_pool(name="expt", bufs=3) as expt,
            tc.tile_pool(name="stats", bufs=10) as stats,
            tc.tile_pool(name="small", bufs=2) as small,
            tc.tile_pool(name="psmm", bufs=2, space="PSUM") as psmm,
            tc.tile_pool(name="pstr", bufs=2, space="PSUM") as pstr,
        ):
            ident = persist.tile([P, P], F32, tag="ident")
            make_identity(nc, ident)
            identr = persist.tile([P, P], F32R, tag="identr")
            nc.vector.tensor_copy(identr, ident)
            for b in range(n_examples):
                wt_sb = big.tile([P, KB, L], F32R, tag="big", name=f"wt_{b}")
                et_sb = big.tile([P, KB, L], F32R, tag="big", name=f"et_{b}")
                srct_sb = big.tile([P, KB, L], F32R, tag="big", name=f"srct_{b}")
                srcz_sb = big.tile([P, KB, L], F32R, tag="big", name=f"srcz_{b}")
                emcol_sb = small.tile([P, KB], F32, tag="emcol", name=f"emc_{b}")
                # tiny first so it never queues behind the 4MB loads
                nc.sync.dma_start(out=emcol_sb, in_=emcol_d[b])
                # additive src-mask row (-1e9 at masked l), replicated
                smrow_rep = small.tile([P, L], F32, tag="s01rep", name=f"s01_{b}")
                s01_src = bass.AP(
                    tensor=s01r_d[b].tensor,
                    offset=s01r_d[b].offset,
                    ap=[[0, P]] + list(s01r_d[b].ap),
                )
                nc.gpsimd.dma_start(out=smrow_rep, in_=s01_src)
                # two HWDGE queues stream in parallel; wt/et k-slices are
                # interleaved across both queues so phase 1's k-serial chain
                # is paced at ~2x a single queue's slice rate
                for k in range(KB):
                    ea, eb = (nc.scalar, nc.sync) if k % 2 == 0 else (nc.sync, nc.scalar)
                    ea.dma_start(out=wt_sb[:, k, :], in_=wt_d[k * P : (k + 1) * P, :].bitcast(F32R))
                    eb.dma_start(out=et_sb[:, k, :], in_=et_d[b][k * P : (k + 1) * P, :].bitcast(F32R))
                load_split(srct_sb, srct_d[b])
                load_split(srcz_sb, srcz_d[b])

                # ---- Phase 1: PT = W @ E^T, laid out (s, e) ----
                # k-outer over groups of 3 chains: each arriving (wt_k, et_k)
                # slice pair feeds 6 matmuls, keeping the PE busy (and its
                # clock ramped) while the loads stream in
                pt_sb = big.tile([P, KB, L], F32R, tag="big", name=f"pt_{b}")
                for g in range(0, KB, 3):
                    sbs = range(g, min(g + 3, KB))
                    pss = {
                        sb: psmm.tile([P, L], F32, tag="psmm", name=f"p1_{b}_{sb}")
                        for sb in sbs
                    }
                    for k in range(KB):
                        for sb in sbs:
                            for n in range(N2):
                                nc.tensor.matmul(
                                    pss[sb][:, ts(n, 512)],
                                    wt_sb[:, k, ts(sb, P)],
                                    et_sb[:, k, ts(n, 512)],
                                    start=(k == 0),
                                    stop=(k == KB - 1),
                                )
                    for sb in sbs:
                        nc.scalar.copy(pt_sb[:, sb, :], pss[sb])

                en_sb = big.tile([P, KB, L], F32R, tag="big", name=f"en_{b}")
                load_split(en_sb, en_d[b])
                # ST accumulates S^T + editmask via PE transposes
                st_sb = big.tile([P, KB, L], F32R, tag="big", name=f"st_{b}")

                # ---- Phase 2: edit path per e-block; also builds ST ----
                # Software-pipelined: the sim chain for mb+1 issues on the PE
                # before block mb's transposes/ctx, so the stats chain for
                # mb+1 (Act/gpsimd/DVE) hides under block mb's PE work.
                def sim_chain(mb):
                    ps = psmm.tile([P, L], F32, tag="psmm", name=f"p2_{b}_{mb}")
                    for n in range(N2):
                        for k in range(KB):
                            nc.tensor.matmul(
                                ps[:, ts(n, 512)],
                                pt_sb[:, k, ts(mb, P)],
                                srct_sb[:, k, ts(n, 512)],
                                start=(k == 0),
                                stop=(k == KB - 1),
                            )
                    return ps

                def stats_edit(mb, ps):
                    # masked copy is the ONLY reader of the psum sim tile, so
                    # its slot frees after one Act op. The per-partition emcol
                    # bias cancels in the softmax shift:
                    # exp(ps - max(ps)) == exp(smask - max(smask)).
                    # src path input: sim + editmask (per-partition bias, Act)
                    smask = blk.tile([P, L], F32R, tag="blk", name=f"smask_{b}_{mb}")
                    nc.scalar.add(smask, ps, add=emcol_sb[:, mb : mb + 1])
                    # edit path input: sim + (-1e9) src-mask row, added on the
                    # DVE straight from PSUM (emcol must NOT leak in here: the
                    # +-1e9 absorption would quantize masked-e rows' sims).
                    # exp() then yields exact zeros at masked l, so Z falls out
                    # of the exp's accumulator.
                    smask2 = blk.tile([P, L], F32, tag="ztmp", name=f"sm2_{b}_{mb}", bufs=2)
                    nc.vector.tensor_add(smask2, ps, smrow_rep)
                    negmax = stats.tile([P, 1], F32, tag="negmax", name=f"nm_{b}_{mb}")
                    nc.vector.reduce_max(negmax, smask2, axis=AX, negate=True)
                    eexp = blk.tile([P, L], F32R, tag="blk", name=f"eexp_{b}_{mb}")
                    zps = stats.tile([P, 1], F32, tag="zl", name=f"ze_{b}_{mb}")
                    nc.scalar.activation(eexp, smask2, EXP, bias=negmax, accum_out=zps)
                    iz = stats.tile([P, 1], F32, tag="iz", name=f"iz_{b}_{mb}")
                    nc.vector.reciprocal(iz, zps)
                    return smask, eexp, iz

                def stats_src(lb):
                    negmax = stats.tile([P, 1], F32, tag="negmax", name=f"nml_{b}_{lb}")
                    nc.vector.reduce_max(negmax, st_sb[:, lb, :], axis=AX, negate=True)
                    zl = stats.tile([P, 1], F32, tag="zl", name=f"zl_{b}_{lb}")
                    nc.scalar.activation(
                        st_sb[:, lb, :], st_sb[:, lb, :], EXP, bias=negmax, accum_out=zl
                    )
                    izl = stats.tile([P, 1], F32, tag="iz", name=f"izl_{b}_{lb}")
                    nc.vector.reciprocal(izl, zl)
                    return izl

                def sw_transposes(lb):
                    sw = expt.tile([P, KB, P], F32R, tag="expt", name=f"sw_{b}_{lb}")
                    for half in range(2):
                        tr = pstr.tile(
                            [P, 4, P], F32R, tag=("tre" if half == 0 else "trm"),
                            name=f"trs_{b}_{lb}_{half}",
                        )
                        for i in range(4):
                            nc.tensor.transpose(
                                tr[:, i, :], st_sb[:, lb, ts(half * 4 + i, P)], identr
                            )
                        if half == 0:
                            nc.scalar.copy(sw[:, 0:4, :], tr)
                        else:
                            nc.vector.tensor_copy(sw[:, 4:8, :], tr)
                    return sw

                iz_q = []
                ps0 = sim_chain(0)
                prev = stats_edit(0, ps0)
                pend = None  # ps of mb+1, stats issued at iteration end
                for mb in range(KB):
                    if mb + 1 < KB:
                        pend = sim_chain(mb + 1)
                    smask, eexp, iz = prev
                    # transposes: eexp quads copied via Act, masked-sim quads
                    # via DVE, so the two staging rings drain in parallel
                    eexpT = expt.tile([P, KB, P], F32R, tag="expt", name=f"eexpT_{b}_{mb}")
                    for half in range(2):
                        trE = pstr.tile([P, 4, P], F32R, tag="tre", name=f"tre_{b}_{mb}_{half}")
                        for i in range(4):
                            nc.tensor.transpose(trE[:, i, :], eexp[:, ts(half * 4 + i, P)], identr)
                        nc.scalar.copy(eexpT[:, half * 4 : (half + 1) * 4, :], trE)
                        trM = pstr.tile([P, 4, P], F32R, tag="trm", name=f"trm_{b}_{mb}_{half}")
                        for i in range(4):
                            nc.tensor.transpose(
                                trM[:, i, :], smask[:, ts(half * 4 + i, P)], identr
                            )
                        nc.vector.tensor_copy(st_sb[:, half * 4 : (half + 1) * 4, ts(mb, P)], trM)
                    # stats for mb+1 issue here (not at iteration end) so the
                    # Act queue runs smask(mb+1) during this ctx chain and the
                    # sim-psum slot is free before sim(mb+2) issues
                    if mb + 1 < KB:
                        nxt = stats_edit(mb + 1, pend)
                    else:
                        # hoist phase-3 lead stats so exp(0)/exp(1) run on the
                        # Act/DVE engines underneath this last ctx chain
                        nxt = None
                        iz_q = [stats_src(0), stats_src(1)]
                    cps = psmm.tile([P, L], F32, tag="psmm", name=f"ec_{b}_{mb}")
                    for n in range(N2):
                        for k in range(KB):
                            nc.tensor.matmul(
                                cps[:, ts(n, 512)],
                                eexpT[:, k, :],
                                srcz_sb[:, k, ts(n, 512)],
                                start=(k == 0),
                                stop=(k == KB - 1),
                            )
                    oe_stage = blk.tile([P, L], F32, tag="blk", name=f"oes_{b}_{mb}")
                    nc.scalar.mul(oe_stage, cps, mul=iz)
                    nc.gpsimd.dma_start(out=oe_d[b, mb * P : (mb + 1) * P, :], in_=oe_stage)
                    prev = nxt

                # ---- Phase 3: src path per l-block, from ST in SBUF ----
                # stats run two ahead (hoisted into phase 2 for lb=0/1) and
                # transposes one ahead, so each iteration's ctx chain starts
                # with its sw weights already staged in SBUF.
                sw_q = [sw_transposes(0)]
                for lb in range(KB):
                    if lb + 1 < KB:
                        sw_q.append(sw_transposes(lb + 1))
                    sw = sw_q.pop(0)
                    cps = psmm.tile([P, L], F32, tag="psmm", name=f"sc_{b}_{lb}")
                    for n in range(N2):
                        for k in range(KB):
                            nc.tensor.matmul(
                                cps[:, ts(n, 512)],
                                sw[:, k, :],
                                en_sb[:, k, ts(n, 512)],
                                start=(k == 0),
                                stop=(k == KB - 1),
                            )
                    os_stage = blk.tile([P, L], F32, tag="blk", name=f"oss_{b}_{lb}")
                    nc.scalar.mul(os_stage, cps, mul=iz_q.pop(0))
                    nc.gpsimd.dma_start(out=os_d[b, lb * P : (lb + 1) * P, :], in_=os_stage)
                    if lb + 2 < KB:
                        iz_q.append(stats_src(lb + 2))

    if fence:
        _fence_matmul_waits(nc)
    return nc


def _fence_matmul_waits(nc):
    """walrus can attach at most one sync wait to the LDWEIGHTS half of a
    self-loading fp32/fp32r matmul. Move every multi-wait Matmult's waits
    onto a PE no-op fence inserted right before it (the NX sequencer
    performs waits in issue order, so the fence strictly precedes the
    matmul's weight read)."""
    f = nc.m.functions[0]
    moved = 0
    for blk in f.blocks:
        out = []
        for inst in blk.instructions:
            si = getattr(inst, "sync_info", None)
            if si is not None and len(si.on_wait) > 1:
                for j, w in enumerate(si.on_wait):
                    nop = mybir.InstNoOp(name=f"{inst.name}-wf{j}", ins=[], outs=[])
                    nop.engine = inst.engine
                    nop.sync_info = mybir.SyncInfo(on_wait=[w], on_update=[])
                    out.append(nop)
                inst.sync_info = mybir.SyncInfo(on_wait=[], on_update=list(si.on_update))
                moved += 1
            out.append(inst)
        blk.instructions = out
    return moved


_NC_CACHE = {}


def get_nc(n_examples=PER_CORE):
    if n_examples not in _NC_CACHE:
        _NC_CACHE[n_examples] = build_nc(n_examples)
    return _NC_CACHE[n_examples]


def make_in_maps(E, S, em, sm, W32):
    """Build the 8 per-core input dicts from full-batch numpy inputs.
    srcz is plain Src: masked-l rows are killed by the exact-zero exp
    weights in the kernel, so no host-side zeroing is needed. The big
    transposes run in a thread pool (numpy releases the GIL)."""
    from concurrent.futures import ThreadPoolExecutor

    wt = np.ascontiguousarray(W32.T)
    with ThreadPoolExecutor(max_workers=8) as pool:
        ets = list(pool.map(lambda b: np.ascontiguousarray(E[b].T), range(B)))
        srcts = list(pool.map(lambda b: np.ascontiguousarray(S[b].T), range(B)))
    in_maps = []
    for c in range(NCORES):
        bs = list(range(PER_CORE * c, PER_CORE * (c + 1)))
        et = np.stack([ets[b] for b in bs])
        srct = np.stack([srcts[b] for b in bs])
        en = np.ascontiguousarray(E[bs])
        emadd = np.where(em[bs] != 0, np.float32(NEG), np.float32(0.0)).astype(np.float32)
        emcol = np.ascontiguousarray(emadd.reshape(PER_CORE, KB, P).transpose(0, 2, 1))
        s01r = np.ascontiguousarray(np.where(sm[bs] != 0, np.float32(NEG), np.float32(0.0)))
        in_maps.append(
            {
                "et": et,
                "srct": srct,
                "srcz": np.ascontiguousarray(S[bs]),
                "en": en,
                "wt": wt,
                "emcol": emcol,
                "s01r": s01r,
            }
        )
    return in_maps


_RUNNER = None


def _get_runner():
    """Build once: nc + a retained jitted SPMD executable. partition_id must
    be bound explicitly or the NEFF waits forever on an unbound input."""
    global _RUNNER
    if _RUNNER is not None:
        return _RUNNER
    import jax
    from jax.sharding import Mesh, PartitionSpec
    from jax.experimental.shard_map import shard_map

    from concourse import bass2jax

    nc = get_nc()
    bass2jax.install_neuronx_cc_hook()
    partition_name = nc.partition_id_tensor.name if nc.partition_id_tensor else None
    in_names, out_names, out_avals, zero_outs = [], [], [], []
    for alloc in nc.m.functions[0].allocations:
        if not isinstance(alloc, mybir.MemoryLocationSet):
            continue
        name = alloc.memorylocations[0].name
        if alloc.kind == "ExternalInput":
            if name != partition_name:
                in_names.append(name)
        elif alloc.kind == "ExternalOutput":
            out_names.append(name)
            shape = tuple(alloc.tensor_shape)
            dtype = mybir.dt.np(alloc.dtype)
            out_avals.append(jax.core.ShapedArray(shape, dtype))
            zero_outs.append(np.zeros(shape, dtype))
    n_params = len(in_names)
    bind_names = list(in_names) + list(out_names)
    if partition_name is not None:
        bind_names = bind_names + [partition_name]

    def _body(*args):
        operands = list(args)
        if partition_name is not None:
            operands.append(bass2jax.partition_id_tensor())
        outs = bass2jax._bass_exec_p.bind(
            *operands,
            out_avals=tuple(out_avals),
            in_names=tuple(bind_names),
            out_names=tuple(out_names),
            lowering_input_output_aliases=(),
            sim_require_finite=True,
            sim_require_nnan=True,
            nc=nc,
        )
        return tuple(outs)

    devices = jax.devices()[:NCORES]
    mesh = Mesh(np.asarray(devices), ("core",))
    n_args = n_params + len(out_names)
    sharded = jax.jit(
        shard_map(
            _body,
            mesh=mesh,
            in_specs=(PartitionSpec("core"),) * n_args,
            out_specs=(PartitionSpec("core"),) * len(out_names),
            check_rep=False,
        ),
        keep_unused=True,
    )
    concat_zeros = [
        np.zeros((NCORES * z.shape[0], *z.shape[1:]), z.dtype) for z in zero_outs
    ]
    _RUNNER = (sharded, in_names, out_names, concat_zeros)
    return _RUNNER


def kernel(edit_encodings, src_encodings, edit_sent_masks, src_sent_masks, W):
    import jax

    E = np.ascontiguousarray(np.asarray(edit_encodings, dtype=np.float32))
    S = np.ascontiguousarray(np.asarray(src_encodings, dtype=np.float32))
    em = np.asarray(edit_sent_masks).astype(np.int32)
    sm = np.asarray(src_sent_masks).astype(np.int32)
    W32 = np.ascontiguousarray(np.asarray(W, dtype=np.float32))

    sharded, in_names, out_names, concat_zeros = _get_runner()
    in_maps = make_in_maps(E, S, em, sm, W32)
    concat_in = [
        np.concatenate([in_maps[c][nm] for c in range(NCORES)], axis=0)
        for nm in in_names
    ]
    out = sharded(*concat_in, *concat_zeros)
    jax.block_until_ready(out)
    oidx = {nm: i for i, nm in enumerate(out_names)}
    edit_ctx = np.asarray(out[oidx["oe"]]).reshape(B, L, L)
    src_ctx = np.asarray(out[oidx["osr"]]).reshape(B, L, L)
    return edit_ctx, src_ctx



# revision 44
# speedup vs baseline: 1.1384x; 1.1384x over previous
"""Trainium2 Bass kernel for the CoAttnLayer problem.

Computes, per example b (B=16, all dims 1024):
    P   = E @ W^T                      (proj)
    S   = P @ Src^T                    (sim, (e, l))
    edit_w  = softmax_l(S + srcmask)   -> edit_ctx = edit_w @ Src
    src_w   = softmax_e(S + editmask)  -> src_ctx  = src_w^T @ E

Sharding: data-parallel over batch, 2 examples per core on 8 cores;
W^T replicated. All heavy matmuls run in float32r (fp32 operands
truncated to ~fp22 by the PE, full fp32 PSUM accumulation).

Key layout trick: the matrix chains are arranged so every softmax
reduction happens along the free dimension, and the exponentiated
weights are transposed on the PE (128x128 transpose-mode matmuls) into
the layout the following context matmul needs as its stationary operand.

Masking (additive -1e9, matching the reference's -inf up to fp32
underflow of exp):
  - edit path (softmax over l): the -1e9 src-mask row is added on the
    DVE straight from the sim PSUM tile (emcol must NOT leak in: its
    +-1e9 would absorb the sims of masked-e rows), so exp() yields
    exact zeros at masked l and Z falls out of the exp accumulator.
    Masked Src rows then contribute 0 x finite to the context matmul,
    so no host-side zeroing of Src is needed.
  - src path (softmax over e): the edit mask is a per-partition Act
    bias on the masked sim copy that the PE transposes into ST.

Schedule (the part that matters for the 309us/core timeline-sim time,
vs 360 for the naive ordering):
  - phase 1 runs k-outer over groups of 3 chains so each arriving
    (wt_k, et_k) DMA slice pair feeds 6 matmuls and the PE clock ramps.
  - phase 2 issues the sim chain for mb+1 on the PE before block mb's
    transposes/ctx; the stats chain for mb+1 (Act add -> DVE max ->
    Act exp) issues between transposes and ctx so the in-order Act
    queue frees the sim-PSUM slot during the ctx chain. PSUM: 2 sim/ctx
    bufs (2 banks each) + 2x2 transpose-staging bufs (1 bank each).
  - phase 3 pipelines stats two ahead (lb=0/1 hoisted under phase 2's
    last ctx) and transposes one ahead.
  - outputs store via the gpsimd SWDGE queue: DMACopy dispatches cost
    ~1us of *sequencer* time, which on the Act HWDGE queue would delay
    the staging copies behind them.
  - the cost model serializes all DMA on one ~358GB/s engine: slice
    loads are ordered by first use (wt/et interleaved on both HWDGE
    queues, then srct, then srcz, then en) and never monolithic, which
    would head-of-line-block later slices.
"""

import os
import sys

for _p in ("/opt/trn_rl_repo",):
    if os.path.isdir(_p) and _p not in sys.path:
        sys.path.insert(0, _p)

import numpy as np

import concourse.bass as bass
import concourse.tile as tile
from concourse import mybir
from concourse.bass import ts
from concourse.masks import make_identity
from concourse.bass_utils import run_bass_kernel_spmd

B = 16
L = 1024          # LE = LS = DE = DS = 1024
NCORES = 8
PER_CORE = B // NCORES
P = 128           # partitions
KB = L // P       # 8 k-tiles
N2 = 2            # two 512-wide n-tiles
F32 = mybir.dt.float32
F32R = mybir.dt.float32r
AX = mybir.AxisListType.X
EXP = mybir.ActivationFunctionType.Exp
NEG = -1.0e9


def build_nc(n_examples=PER_CORE, fence=True):
    nc = bass.Bass("TRN2", target_bir_lowering=False)

    et_d = nc.dram_tensor("et", (n_examples, L, L), F32, kind="ExternalInput")     # E^T  (d, e)
    srct_d = nc.dram_tensor("srct", (n_examples, L, L), F32, kind="ExternalInput")  # Src^T (s, l)
    srcz_d = nc.dram_tensor("srcz", (n_examples, L, L), F32, kind="ExternalInput")  # Src, masked rows zeroed (l, s)
    en_d = nc.dram_tensor("en", (n_examples, L, L), F32, kind="ExternalInput")      # E natural (e, d)
    wt_d = nc.dram_tensor("wt", (L, L), F32, kind="ExternalInput")                  # W^T (d, s)
    emcol_d = nc.dram_tensor("emcol", (n_examples, P, KB), F32, kind="ExternalInput")  # additive edit mask [p, eb]
    s01r_d = nc.dram_tensor("s01r", (n_examples, L), F32, kind="ExternalInput")     # src validity 0/1 row
    oe_d = nc.dram_tensor("oe", (n_examples, L, L), F32, kind="ExternalOutput")     # edit_ctx
    os_d = nc.dram_tensor("osr", (n_examples, L, L), F32, kind="ExternalOutput")    # src_ctx

    def load_split(sb_tile, dram, eng=None):
        # one dma_start per k-slice so consumers start on the first slice
        # instead of the whole 4MB tensor; eng picks the HWDGE queue
        e = eng if eng is not None else nc.sync
        for k in range(KB):
            e.dma_start(
                out=sb_tile[:, k, :],
                in_=dram[k * P : (k + 1) * P, :].bitcast(F32R),
            )

    def load_one(sb_tile, dram, eng=None):
        # whole tensor in one DMA dispatch (queue-sequencer time is the
        # scarce resource, not DMA bandwidth)
        e = eng if eng is not None else nc.sync
        e.dma_start(
            out=sb_tile,
            in_=dram.rearrange("(k p) l -> p k l", p=P).bitcast(F32R),
        )

    with tile.TileContext(nc) as tc:
        with (
            tc.tile_pool(name="persist", bufs=1) as persist,
            tc.tile_pool(name="big", bufs=5) as big,
            tc.tile_pool(name="blk", bufs=4) as blk,
            tc.tile_pool(name="expt", bufs=3) as expt,
            tc.tile_pool(name="stats", bufs=10) as stats,
            tc.tile_pool(name="small", bufs=2) as small,
            tc.tile_pool(name="psmm", bufs=2, space="PSUM") as psmm,
            tc.tile_pool(name="pstr", bufs=2, space="PSUM") as pstr,
        ):
            ident = persist.tile([P, P], F32, tag="ident")
            make_identity(nc, ident)
            identr = persist.tile([P, P], F32R, tag="identr")
            nc.vector.tensor_copy(identr, ident)
            for b in range(n_examples):
                wt_sb = big.tile([P, KB, L], F32R, tag="big", name=f"wt_{b}")
                et_sb = big.tile([P, KB, L], F32R, tag="big", name=f"et_{b}")
                srct_sb = big.tile([P, KB, L], F32R, tag="big", name=f"srct_{b}")
                srcz_sb = big.tile([P, KB, L], F32R, tag="big", name=f"srcz_{b}")
                emcol_sb = small.tile([P, KB], F32, tag="emcol", name=f"emc_{b}")
                # tiny first so it never queues behind the 4MB loads
                nc.sync.dma_start(out=emcol_sb, in_=emcol_d[b])
                # additive src-mask row (-1e9 at masked l), replicated
                smrow_rep = small.tile([P, L], F32, tag="s01rep", name=f"s01_{b}")
                s01_src = bass.AP(
                    tensor=s01r_d[b].tensor,
                    offset=s01r_d[b].offset,
                    ap=[[0, P]] + list(s01r_d[b].ap),
                )
                nc.gpsimd.dma_start(out=smrow_rep, in_=s01_src)
                # two HWDGE queues stream in parallel; wt/et k-slices are
                # interleaved across both queues so phase 1's k-serial chain
                # is paced at ~2x a single queue's slice rate
                for k in range(KB):
                    ea, eb = (nc.scalar, nc.sync) if k % 2 == 0 else (nc.sync, nc.scalar)
                    ea.dma_start(out=wt_sb[:, k, :], in_=wt_d[k * P : (k + 1) * P, :].bitcast(F32R))
                    eb.dma_start(out=et_sb[:, k, :], in_=et_d[b][k * P : (k + 1) * P, :].bitcast(F32R))
                load_split(srct_sb, srct_d[b])
                load_split(srcz_sb, srcz_d[b])

                # ---- Phase 1: PT = W @ E^T, laid out (s, e) ----
                # k-outer over groups of 3 chains: each arriving (wt_k, et_k)
                # slice pair feeds 6 matmuls, keeping the PE busy (and its
                # clock ramped) while the loads stream in
                pt_sb = big.tile([P, KB, L], F32R, tag="big", name=f"pt_{b}")
                for g in range(0, KB, 3):
                    sbs = range(g, min(g + 3, KB))
                    pss = {
                        sb: psmm.tile([P, L], F32, tag="psmm", name=f"p1_{b}_{sb}")
                        for sb in sbs
                    }
                    for k in range(KB):
                        for sb in sbs:
                            for n in range(N2):
                                nc.tensor.matmul(
                                    pss[sb][:, ts(n, 512)],
                                    wt_sb[:, k, ts(sb, P)],
                                    et_sb[:, k, ts(n, 512)],
                                    start=(k == 0),
                                    stop=(k == KB - 1),
                                )
                    for sb in sbs:
                        nc.scalar.copy(pt_sb[:, sb, :], pss[sb])

                en_sb = big.tile([P, KB, L], F32R, tag="big", name=f"en_{b}")
                load_split(en_sb, en_d[b])
                # ST accumulates S^T + editmask via PE transposes
                st_sb = big.tile([P, KB, L], F32R, tag="big", name=f"st_{b}")

                # ---- Phase 2: edit path per e-block; also builds ST ----
                # Software-pipelined: the sim chain for mb+1 issues on the PE
                # before block mb's transposes/ctx, so the stats chain for
                # mb+1 (Act/gpsimd/DVE) hides under block mb's PE work.
                def sim_chain(mb):
                    ps = psmm.tile([P, L], F32, tag="psmm", name=f"p2_{b}_{mb}")
                    for n in range(N2):
                        for k in range(KB):
                            nc.tensor.matmul(
                                ps[:, ts(n, 512)],
                                pt_sb[:, k, ts(mb, P)],
                                srct_sb[:, k, ts(n, 512)],
                                start=(k == 0),
                                stop=(k == KB - 1),
                            )
                    return ps

                def stats_edit_a(mb, ps):
                    # DVE leg, issued right after the sim chain so it is NOT
                    # queued behind the trm staging copies: sim + (-1e9)
                    # src-mask row straight from PSUM (emcol must NOT leak in
                    # here: its +-1e9 absorption would quantize masked-e rows'
                    # sims), then the row max.
                    smask2 = blk.tile([P, L], F32, tag="ztmp", name=f"sm2_{b}_{mb}", bufs=2)
                    nc.vector.tensor_add(smask2, ps, smrow_rep)
                    negmax = stats.tile([P, 1], F32, tag="negmax", name=f"nm_{b}_{mb}")
                    nc.vector.reduce_max(negmax, smask2, axis=AX, negate=True)
                    return smask2, negmax

                def stats_edit_b(mb, ps, a):
                    # Act leg: masked copy for the src path (2nd reader of the
                    # psum tile) and the exp, whose accumulator is Z (exp is
                    # exactly zero at masked l).
                    smask2, negmax = a
                    smask = blk.tile([P, L], F32R, tag="blk", name=f"smask_{b}_{mb}")
                    nc.scalar.add(smask, ps, add=emcol_sb[:, mb : mb + 1])
                    eexp = blk.tile([P, L], F32R, tag="blk", name=f"eexp_{b}_{mb}")
                    zps = stats.tile([P, 1], F32, tag="zl", name=f"ze_{b}_{mb}")
                    nc.scalar.activation(eexp, smask2, EXP, bias=negmax, accum_out=zps)
                    iz = stats.tile([P, 1], F32, tag="iz", name=f"iz_{b}_{mb}")
                    nc.vector.reciprocal(iz, zps)
                    return smask, eexp, iz

                def stats_src(lb):
                    negmax = stats.tile([P, 1], F32, tag="negmax", name=f"nml_{b}_{lb}")
                    nc.vector.reduce_max(negmax, st_sb[:, lb, :], axis=AX, negate=True)
                    zl = stats.tile([P, 1], F32, tag="zl", name=f"zl_{b}_{lb}")
                    nc.scalar.activation(
                        st_sb[:, lb, :], st_sb[:, lb, :], EXP, bias=negmax, accum_out=zl
                    )
                    izl = stats.tile([P, 1], F32, tag="iz", name=f"izl_{b}_{lb}")
                    nc.vector.reciprocal(izl, zl)
                    return izl

                def sw_transposes(lb):
                    sw = expt.tile([P, KB, P], F32R, tag="expt", name=f"sw_{b}_{lb}")
                    for half in range(2):
                        tr = pstr.tile(
                            [P, 4, P], F32R, tag=("tre" if half == 0 else "trm"),
                            name=f"trs_{b}_{lb}_{half}",
                        )
                        for i in range(4):
                            nc.tensor.transpose(
                                tr[:, i, :], st_sb[:, lb, ts(half * 4 + i, P)], identr
                            )
                        if half == 0:
                            nc.scalar.copy(sw[:, 0:4, :], tr)
                        else:
                            nc.vector.tensor_copy(sw[:, 4:8, :], tr)
                    return sw

                iz_q = []
                ps0 = sim_chain(0)
                prev = stats_edit_b(0, ps0, stats_edit_a(0, ps0))
                pend = None  # ps of mb+1; DVE stats issue right after its sim
                pend_a = None
                for mb in range(KB):
                    if mb + 1 < KB:
                        pend = sim_chain(mb + 1)
                        pend_a = stats_edit_a(mb + 1, pend)
                    smask, eexp, iz = prev
                    # transposes: eexp quads copied via Act, masked-sim quads
                    # via DVE, so the two staging rings drain in parallel
                    eexpT = expt.tile([P, KB, P], F32R, tag="expt", name=f"eexpT_{b}_{mb}")
                    for half in range(2):
                        trE = pstr.tile([P, 4, P], F32R, tag="tre", name=f"tre_{b}_{mb}_{half}")
                        for i in range(4):
                            nc.tensor.transpose(trE[:, i, :], eexp[:, ts(half * 4 + i, P)], identr)
                        nc.scalar.copy(eexpT[:, half * 4 : (half + 1) * 4, :], trE)
                        trM = pstr.tile([P, 4, P], F32R, tag="trm", name=f"trm_{b}_{mb}_{half}")
                        for i in range(4):
                            nc.tensor.transpose(
                                trM[:, i, :], smask[:, ts(half * 4 + i, P)], identr
                            )
                        nc.vector.tensor_copy(st_sb[:, half * 4 : (half + 1) * 4, ts(mb, P)], trM)
                    # stats for mb+1 issue here (not at iteration end) so the
                    # Act queue runs smask(mb+1) during this ctx chain and the
                    # sim-psum slot is free before sim(mb+2) issues
                    if mb + 1 < KB:
                        nxt = stats_edit_b(mb + 1, pend, pend_a)
                    else:
                        # hoist phase-3 lead stats so exp(0)/exp(1) run on the
                        # Act/DVE engines underneath this last ctx chain
                        nxt = None
                        iz_q = [stats_src(0), stats_src(1)]
                    cps = psmm.tile([P, L], F32, tag="psmm", name=f"ec_{b}_{mb}")
                    for n in range(N2):
                        for k in range(KB):
                            nc.tensor.matmul(
                                cps[:, ts(n, 512)],
                                eexpT[:, k, :],
                                srcz_sb[:, k, ts(n, 512)],
                                start=(k == 0),
                                stop=(k == KB - 1),
                            )
                    oe_stage = blk.tile([P, L], F32, tag="blk", name=f"oes_{b}_{mb}")
                    nc.scalar.mul(oe_stage, cps, mul=iz)
                    nc.gpsimd.dma_start(out=oe_d[b, mb * P : (mb + 1) * P, :], in_=oe_stage)
                    prev = nxt

                # ---- Phase 3: src path per l-block, from ST in SBUF ----
                # stats run two ahead (hoisted into phase 2 for lb=0/1) and
                # transposes one ahead, so each iteration's ctx chain starts
                # with its sw weights already staged in SBUF.
                sw_q = [sw_transposes(0)]
                for lb in range(KB):
                    if lb + 1 < KB:
                        sw_q.append(sw_transposes(lb + 1))
                    sw = sw_q.pop(0)
                    cps = psmm.tile([P, L], F32, tag="psmm", name=f"sc_{b}_{lb}")
                    for n in range(N2):
                        for k in range(KB):
                            nc.tensor.matmul(
                                cps[:, ts(n, 512)],
                                sw[:, k, :],
                                en_sb[:, k, ts(n, 512)],
                                start=(k == 0),
                                stop=(k == KB - 1),
                            )
                    os_stage = blk.tile([P, L], F32, tag="blk", name=f"oss_{b}_{lb}")
                    izv = iz_q.pop(0)
                    if lb == KB - 1:
                        # tail: scale+store in halves so the drain barrier
                        # isn't gated on one full mul + one full transfer
                        nc.scalar.mul(os_stage[:, 0:512], cps[:, 0:512], mul=izv)
                        nc.gpsimd.dma_start(out=os_d[b, lb * P : (lb + 1) * P, 0:512], in_=os_stage[:, 0:512])
                        nc.scalar.mul(os_stage[:, 512:1024], cps[:, 512:1024], mul=izv)
                        nc.gpsimd.dma_start(out=os_d[b, lb * P : (lb + 1) * P, 512:1024], in_=os_stage[:, 512:1024])
                    else:
                        nc.scalar.mul(os_stage, cps, mul=izv)
                        nc.gpsimd.dma_start(out=os_d[b, lb * P : (lb + 1) * P, :], in_=os_stage)
                    if lb + 2 < KB:
                        iz_q.append(stats_src(lb + 2))

    if fence:
        _fence_matmul_waits(nc)
    return nc


def _fence_matmul_waits(nc):
    """walrus can attach at most one sync wait to the LDWEIGHTS half of a
    self-loading fp32/fp32r matmul. Move every multi-wait Matmult's waits
    onto a PE no-op fence inserted right before it (the NX sequencer
    performs waits in issue order, so the fence strictly precedes the
    matmul's weight read)."""
    f = nc.m.functions[0]
    moved = 0
    for blk in f.blocks:
        out = []
        for inst in blk.instructions:
            si = getattr(inst, "sync_info", None)
            if si is not None and len(si.on_wait) > 1:
                for j, w in enumerate(si.on_wait):
                    nop = mybir.InstNoOp(name=f"{inst.name}-wf{j}", ins=[], outs=[])
                    nop.engine = inst.engine
                    nop.sync_info = mybir.SyncInfo(on_wait=[w], on_update=[])
                    out.append(nop)
                inst.sync_info = mybir.SyncInfo(on_wait=[], on_update=list(si.on_update))
                moved += 1
            out.append(inst)
        blk.instructions = out
    return moved


_NC_CACHE = {}


def get_nc(n_examples=PER_CORE):
    if n_examples not in _NC_CACHE:
        _NC_CACHE[n_examples] = build_nc(n_examples)
    return _NC_CACHE[n_examples]


def make_in_maps(E, S, em, sm, W32):
    """Build the 8 per-core input dicts from full-batch numpy inputs.
    srcz is plain Src: masked-l rows are killed by the exact-zero exp
    weights in the kernel, so no host-side zeroing is needed. The big
    transposes run in a thread pool (numpy releases the GIL)."""
    from concurrent.futures import ThreadPoolExecutor

    wt = np.ascontiguousarray(W32.T)
    with ThreadPoolExecutor(max_workers=8) as pool:
        ets = list(pool.map(lambda b: np.ascontiguousarray(E[b].T), range(B)))
        srcts = list(pool.map(lambda b: np.ascontiguousarray(S[b].T), range(B)))
    in_maps = []
    for c in range(NCORES):
        bs = list(range(PER_CORE * c, PER_CORE * (c + 1)))
        et = np.stack([ets[b] for b in bs])
        srct = np.stack([srcts[b] for b in bs])
        en = np.ascontiguousarray(E[bs])
        emadd = np.where(em[bs] != 0, np.float32(NEG), np.float32(0.0)).astype(np.float32)
        emcol = np.ascontiguousarray(emadd.reshape(PER_CORE, KB, P).transpose(0, 2, 1))
        s01r = np.ascontiguousarray(np.where(sm[bs] != 0, np.float32(NEG), np.float32(0.0)))
        in_maps.append(
            {
                "et": et,
                "srct": srct,
                "srcz": np.ascontiguousarray(S[bs]),
                "en": en,
                "wt": wt,
                "emcol": emcol,
                "s01r": s01r,
            }
        )
    return in_maps


_RUNNER = None


def _get_runner():
    """Build once: nc + a retained jitted SPMD executable. partition_id must
    be bound explicitly or the NEFF waits forever on an unbound input."""
    global _RUNNER
    if _RUNNER is not None:
        return _RUNNER
    import jax
    from jax.sharding import Mesh, PartitionSpec
    from jax.experimental.shard_map import shard_map

    from concourse import bass2jax

    nc = get_nc()
    bass2jax.install_neuronx_cc_hook()
    partition_name = nc.partition_id_tensor.name if nc.partition_id_tensor else None
    in_names, out_names, out_avals, zero_outs = [], [], [], []
    for alloc in nc.m.functions[0].allocations:
        if not isinstance(alloc, mybir.MemoryLocationSet):
            continue
        name = alloc.memorylocations[0].name
        if alloc.kind == "ExternalInput":
            if name != partition_name:
                in_names.append(name)
        elif alloc.kind == "ExternalOutput":
            out_names.append(name)
            shape = tuple(alloc.tensor_shape)
            dtype = mybir.dt.np(alloc.dtype)
            out_avals.append(jax.core.ShapedArray(shape, dtype))
            zero_outs.append(np.zeros(shape, dtype))
    n_params = len(in_names)
    bind_names = list(in_names) + list(out_names)
    if partition_name is not None:
        bind_names = bind_names + [partition_name]

    def _body(*args):
        operands = list(args)
        if partition_name is not None:
            operands.append(bass2jax.partition_id_tensor())
        outs = bass2jax._bass_exec_p.bind(
            *operands,
            out_avals=tuple(out_avals),
            in_names=tuple(bind_names),
            out_names=tuple(out_names),
            lowering_input_output_aliases=(),
            sim_require_finite=True,
            sim_require_nnan=True,
            nc=nc,
        )
        return tuple(outs)

    devices = jax.devices()[:NCORES]
    mesh = Mesh(np.asarray(devices), ("core",))
    n_args = n_params + len(out_names)
    sharded = jax.jit(
        shard_map(
            _body,
            mesh=mesh,
            in_specs=(PartitionSpec("core"),) * n_args,
            out_specs=(PartitionSpec("core"),) * len(out_names),
            check_rep=False,
        ),
        keep_unused=True,
    )
    concat_zeros = [
        np.zeros((NCORES * z.shape[0], *z.shape[1:]), z.dtype) for z in zero_outs
    ]
    _RUNNER = (sharded, in_names, out_names, concat_zeros)
    return _RUNNER


def kernel(edit_encodings, src_encodings, edit_sent_masks, src_sent_masks, W):
    import jax

    E = np.ascontiguousarray(np.asarray(edit_encodings, dtype=np.float32))
    S = np.ascontiguousarray(np.asarray(src_encodings, dtype=np.float32))
    em = np.asarray(edit_sent_masks).astype(np.int32)
    sm = np.asarray(src_sent_masks).astype(np.int32)
    W32 = np.ascontiguousarray(np.asarray(W, dtype=np.float32))

    sharded, in_names, out_names, concat_zeros = _get_runner()
    in_maps = make_in_maps(E, S, em, sm, W32)
    concat_in = [
        np.concatenate([in_maps[c][nm] for c in range(NCORES)], axis=0)
        for nm in in_names
    ]
    out = sharded(*concat_in, *concat_zeros)
    jax.block_until_ready(out)
    oidx = {nm: i for i, nm in enumerate(out_names)}
    edit_ctx = np.asarray(out[oidx["oe"]]).reshape(B, L, L)
    src_ctx = np.asarray(out[oidx["osr"]]).reshape(B, L, L)
    return edit_ctx, src_ctx

